# revision 1
# baseline (speedup 1.0000x reference)
"""MoE layer (router + top-k dispatch + per-expert FFN + weighted combine)
on 8 Trainium2 NeuronCores.

Sharding strategy (expert-parallel, host-side dispatch + combine):
  - Core e owns expert e's weights (W1[e], W2[e], b1[e], b2[e]).
  - The host computes the router (x @ Wg -> softmax -> top-k) to decide
    WHICH tokens go to which core (the dispatch step of the sharding),
    gathers each expert's tokens, and ships them transposed token-minor so
    both FFN GEMMs run with contraction on the partition axis and zero
    on-device transposes.
  - Device output is yT = (relu(W1^T x + b1))^T W2 + b2, transposed [O, C] —
    the device does ONLY the two dense GEMMs; the softmax combine weight
    probs[token, e] is applied on the host during the unshard scatter-add
    (the "weighted return" half of the expert-parallel all-to-all).

Layout: the host pre-permutes each tensor into the exact per-partition
byte order the PE consumes it in:
  - w1 as [128p, mh, kd, 128q]-flat  (GEMM1 group order mh-major),
  - w2 as [128p, mo, kh, 128q]-flat  (GEMM2 group order mo-major),
  - x  as [128p, chunk, kd, col]-flat (chunk-major).
Every weight/x DMA is then a flat contiguous slice (>=4 KiB per-partition
lines, near-peak HBM efficiency) that arrives in consumption order, so
GEMM1 chunk 0 never waits on a strided straggler.

Compute is bf16 (fp32 PSUM accumulation); combine weights stay fp32.
"""

import numpy as np
import ml_dtypes
import bass_rust

import concourse.bass as bass
import concourse.mybir as mybir
import concourse.tile as tile
from concourse.bass_utils import run_bass_kernel_spmd

P = 128
N_CORES = 8
CHUNK = 512

def _normalize_sync_waits(nc):
    """The walrus build in this toolchain rejects >1 sync wait on a single
    instruction (setupSyncWait: "Too many sync wait commands"), while Tile's
    semaphore assignment freely emits several. Hoist all but one wait of each
    instruction onto same-engine NOPs placed immediately before it — the
    engine stream is in-order, so stalling at the NOPs is semantically
    identical to a multi-wait instruction."""
    count = 0
    for f in nc.m.functions:
        for bb in f.blocks:
            out = []
            changed = False
            for ins in bb.instructions:
                si = ins.sync_info
                if si is not None and len(si.on_wait) > 1:
                    waits = list(si.on_wait)
                    for w in waits[:-1]:
                        count += 1
                        out.append(
                            mybir.InstNoOp(
                                name=f"I-nw{count}",
                                ins=[],
                                outs=[],
                                engine=ins.engine,
                                sync_info=bass_rust.SyncInfo(
                                    on_wait=[w], on_update=[]
                                ),
                            )
                        )
                    ins.sync_info = bass_rust.SyncInfo(
                        on_wait=[waits[-1]], on_update=list(si.on_update)
                    )
                    changed = True
                out.append(ins)
            if changed:
                bb.instructions = out
    return nc


def _build_program(D, H, O, C, chunks):
    f32, bf16 = mybir.dt.float32, mybir.dt.bfloat16
    KD, MH, MO = D // P, H // P, O // P
    AF = mybir.ActivationFunctionType

    nc = bass.Bass()
    xTp = nc.declare_dram_parameter("xTp", [P, KD * C], bf16, isOutput=False)
    w1p = nc.declare_dram_parameter("w1p", [P, MH * KD * P], bf16, isOutput=False)
    w2p = nc.declare_dram_parameter("w2p", [P, MO * MH * P], bf16, isOutput=False)
    b1p = nc.declare_dram_parameter("b1p", [P, MH], f32, isOutput=False)
    b2p = nc.declare_dram_parameter("b2p", [P, MO], f32, isOutput=False)
    yT = nc.declare_dram_parameter("yT", [O, C], f32, isOutput=True)

    with tile.TileContext(nc) as tc:
        with (
            tc.tile_pool(name="sb", bufs=1) as pool,
            tc.tile_pool(name="ps", bufs=4, space="PSUM") as psp,
        ):
            w1_sb = pool.tile([P, MH * KD * P], bf16)
            w2_sb = pool.tile([P, MO * MH * P], bf16)
            b1_sb = pool.tile([P, MH], f32)
            b2_sb = pool.tile([P, MO], f32)
            xc0 = pool.tile([P, KD * CHUNK], bf16, tag="xc", bufs=2)
            N0 = chunks[0]
            GU = KD * P  # flat w1 columns per GEMM1 group

            # Cold start: only gpsimd/SP/Activation queues can trigger DMAs
            # (~0.6-1us of queue time each) and the first ~30us are
            # HBM-wire-bound, so w1 rides as a few flat slices cut at GEMM1
            # group boundaries — arrival order == consumption order — while
            # the scalar queue stays clear for the evictions.  Tile RAW deps
            # are per-DMA-region, so group g only waits for its own slice.
            nc.scalar.dma_start(b1_sb[:], b1p[:])
            nc.sync.dma_start(xc0[:, : 2 * N0], xTp[:, : 2 * N0])
            nc.gpsimd.dma_start(xc0[:, 2 * N0 : KD * N0], xTp[:, 2 * N0 : KD * N0])
            nc.scalar.dma_start(b2_sb[:], b2p[:])
            w1_cuts = [(0, 2), (2, 4), (4, 8), (8, 12), (12, 16), (16, 24), (24, 32)]
            for i, (g0, g1) in enumerate(w1_cuts):
                eng = nc.sync if i % 2 == 0 else nc.gpsimd
                eng.dma_start(
                    w1_sb[:, g0 * GU : g1 * GU], w1p[:, g0 * GU : g1 * GU]
                )

            offs = [sum(chunks[:i]) for i in range(len(chunks))]

            def emit_gemms(ci, xc, hooks={}):
                N, c0 = chunks[ci], offs[ci]
                # GEMM1: h^T = relu(W1^T @ x^T + b1), evicted to SBUF as bf16.
                # h is split into two half-tiles so the next chunk's GEMM1 can
                # start evicting into the first half as soon as this chunk's
                # GEMM2 has consumed it (tile deps are per-tile, not
                # per-region) — removes the chunk-boundary WAW bubble.
                hT_a = pool.tile([P, MH // 2, CHUNK], bf16, tag="h_a")
                hT_b = pool.tile([P, MH // 2, CHUNK], bf16, tag="h_b")

                def h_slice(kh, N=N, hT_a=hT_a, hT_b=hT_b):
                    t = hT_a if kh < MH // 2 else hT_b
                    return t[:, kh % (MH // 2), :N]

                for mh in range(MH):
                    ph = psp.tile([P, CHUNK], f32, tag="ph")
                    for kd in range(KD):
                        u = mh * KD + kd
                        nc.tensor.matmul(
                            ph[:, :N],
                            w1_sb[:, u * P : (u + 1) * P],
                            xc[:, kd * N : (kd + 1) * N],
                            start=(kd == 0),
                            stop=(kd == KD - 1),
                        )
                    nc.scalar.activation(
                        h_slice(mh), ph[:, :N], AF.Relu, bias=b1_sb[:, mh : mh + 1]
                    )
                    if mh in hooks:
                        hooks[mh]()

                # GEMM2: y^T = W2^T @ h^T + b2, evicted straight to DRAM; the
                # softmax combine weight is applied host-side at unshard.
                for mo in range(MO):
                    py = psp.tile([P, CHUNK], f32, tag="py")
                    for kh in range(MH):
                        u = mo * MH + kh
                        nc.tensor.matmul(
                            py[:, :N],
                            w2_sb[:, u * P : (u + 1) * P],
                            h_slice(kh),
                            start=(kh == 0),
                            stop=(kh == MH - 1),
                        )
                    ob = pool.tile([P, CHUNK], f32, tag="ob", bufs=4)
                    nc.scalar.activation(
                        ob[:, :N], py[:, :N], AF.Identity, bias=b2_sb[:, mo : mo + 1]
                    )
                    nc.sync.dma_start(yT[mo * P : (mo + 1) * P, c0 : c0 + N], ob[:, :N])

            # w2 (8.4 MB) would otherwise start transferring early and steal
            # HBM bandwidth from the w1 stream GEMM1 chunk 0 is consuming.
            # Gate each half behind a one-element scalar write into its
            # destination region, emitted after GEMM1 group 10/14's
            # eviction: the in-order gpsimd queue then holds the trigger
            # until chunk-0 compute reaches that group (~29/36us) — the wire
            # is free by then and both halves still land well before GEMM2
            # chunk 0 (~66us).
            W2HALF = (MO // 2) * MH * P

            def gated_w2_half(j0):
                def fn():
                    nc.scalar.activation(
                        w2_sb[:, j0 : j0 + 1], b1_sb[:, 0:1], AF.Copy
                    )
                    nc.gpsimd.dma_start(
                        w2_sb[:, j0 : j0 + W2HALF], w2p[:, j0 : j0 + W2HALF]
                    )
                return fn

            hooks0 = {10: gated_w2_half(0), 14: gated_w2_half(W2HALF)}

            for ci in range(len(chunks)):
                if ci + 1 < len(chunks):
                    N1, c1 = chunks[ci + 1], offs[ci + 1]
                    xc_next = pool.tile([P, KD * CHUNK], bf16, tag="xc", bufs=2)
                    nc.gpsimd.dma_start(
                        xc_next[:, : KD * N1], xTp[:, KD * c1 : KD * (c1 + N1)]
                    )
                emit_gemms(ci, xc0, hooks0 if ci == 0 else {})
                xc0 = xc_next if ci + 1 < len(chunks) else None
    return _normalize_sync_waits(nc)


def kernel(**inputs):
    x = np.ascontiguousarray(np.asarray(inputs["x"], dtype=np.float32))
    Wg = np.ascontiguousarray(np.asarray(inputs["Wg"], dtype=np.float32))
    W1 = np.asarray(inputs["W1"], dtype=np.float32)
    b1 = np.asarray(inputs["b1"], dtype=np.float32)
    W2 = np.asarray(inputs["W2"], dtype=np.float32)
    b2 = np.asarray(inputs["b2"], dtype=np.float32)
    k = int(np.asarray(inputs["k"]))

    B, D = x.shape
    E = Wg.shape[1]
    H = W1.shape[2]
    O = W2.shape[2]
    KD, MH, MO = D // P, H // P, O // P
    assert E == N_CORES, f"expert-per-core layout expects E == 8, got {E}"

    # Host-side router: logits -> softmax probs (combine weights) and top-k
    # expert membership (softmax is monotonic, so top-k on logits == top-k
    # on probs).
    logits = x @ Wg
    m = logits.max(axis=1, keepdims=True)
    p = np.exp(logits - m)
    probs = p / p.sum(axis=1, keepdims=True)
    kth = np.partition(logits, E - k, axis=1)[:, E - k]  # k-th largest per token
    routed = logits >= kth[:, None]  # [B, E] membership mask
    idx_per_e = [np.nonzero(routed[:, e])[0] for e in range(E)]
    counts = [len(ix) for ix in idx_per_e]

    # Capacity: pad the largest expert's token count to a multiple of 8.
    # Split into <=512-token chunks; keep every chunk >=256 (below that the
    # fixed per-matmul issue/LDWEIGHTS cost stops amortizing) by borrowing
    # from the previous full chunk.
    C = max(CHUNK, -(-max(counts) // 8) * 8)
    chunks = [CHUNK] * (C // CHUNK)
    rem = C % CHUNK
    if rem:
        if rem < 256 and chunks:
            chunks[-1] -= 256 - rem
            rem = 256
        chunks.append(rem)
    offs = [sum(chunks[:i]) for i in range(len(chunks))]

    nc = _build_program(D, H, O, C, chunks)

    # Pre-permute every tensor into the flat per-partition consumption-order
    # layout the device program expects (see module docstring).
    in_maps = []
    for e in range(E):
        idx = idx_per_e[e]
        pad = np.zeros(C, dtype=np.int64)
        pad[: counts[e]] = idx
        xg = x[pad].astype(ml_dtypes.bfloat16)  # [C, D]
        xparts = []
        for N, c0 in zip(chunks, offs):
            seg = xg[c0 : c0 + N].T  # [D, N]
            xparts.append(
                seg.reshape(KD, P, N).transpose(1, 0, 2).reshape(P, KD * N)
            )
        xTp_e = np.ascontiguousarray(np.concatenate(xparts, axis=1))
        w1p_e = np.ascontiguousarray(
            W1[e]
            .reshape(KD, P, MH, P)
            .transpose(1, 2, 0, 3)
            .reshape(P, MH * KD * P)
            .astype(ml_dtypes.bfloat16)
        )
        w2p_e = np.ascontiguousarray(
            W2[e]
            .reshape(MH, P, MO, P)
            .transpose(1, 2, 0, 3)
            .reshape(P, MO * MH * P)
            .astype(ml_dtypes.bfloat16)
        )
        in_maps.append(
            {
                "xTp": xTp_e,
                "w1p": w1p_e,
                "w2p": w2p_e,
                "b1p": np.ascontiguousarray(b1[e].reshape(MH, P).T),
                "b2p": np.ascontiguousarray(b2[e].reshape(MO, P).T),
            }
        )

    res = run_bass_kernel_spmd(nc, in_maps, core_ids=list(range(N_CORES)))
    globals()["_last_results"] = res

    out = np.zeros((B, O), dtype=np.float32)
    for e in range(E):
        cnt = counts[e]
        if cnt:
            idx = idx_per_e[e]
            yT_e = res.results[e]["yT"]
            out[idx] += probs[idx, e : e + 1] * yT_e[:, :cnt].T
    return out

